# revision 29
# baseline (speedup 1.0000x reference)
"""GAT (2-layer, PyG-style) distributed Bass kernel for 8 TRN2 NeuronCores.

Strategy (sharding_hint: 1D node partition by dst):
  - core c owns dst nodes [c*NPC, (c+1)*NPC)
  - dense phase: each core computes table1 = [h1(64) | a_src1(8)] rows for its
    node slice (x_slice @ [W1 | W1@A1s | W1@A1d]), AllGather -> full table in
    every core's DRAM.
  - edge phase: host packs edges into "windows" (<=16 dst nodes, <=128 edges;
    1 window == 1 tile of 128 edge slots; 8 windows = 1 block of 128 node
    positions).  Per tile: indirect-DMA gather of source rows + a_dst rows,
    softmax weights w = exp(leakyrelu(a_src+a_dst)) on-chip (no max
    subtraction; |logit| < ~3 verified), then ONE TensorE matmul per tile:
      blk[pos, 0:64 | 64:72] += S_pos^T @ [ h1*w_head | w ]
    where S_pos is the one-hot (edge -> window-pair position) matrix as lhsT
    (col-tiled, tile_position=(0,32p)).  Denominators land in cols 64:72.
  - block post: normalize per head, +bias, ELU, transpose once, W2aug matmul
    -> layer-2 table rows [h2(40) | a_src2(1)] + a_dst2, scattered to DRAM by
    node index (OOB rows skipped); AllGather #2; same edge pass for layer 2;
    log_softmax; scatter rows to the output.
All floating-point math runs on-device.  Host work is integer graph
preprocessing (sort/pack/index-building) and weight layout rearrangement.
"""
import os
import sys
import numpy as np

try:
    import concourse.bass as bass
except ImportError:  # pragma: no cover
    for p in ("/opt/trn_rl_repo", "/root/.axon_site/_ro/trn_rl_repo"):
        if os.path.isdir(p) and p not in sys.path:
            sys.path.insert(0, p)
    import concourse.bass as bass

import ml_dtypes
import concourse.mybir as mybir
import concourse.tile as tile
import concourse.bacc as bacc
from concourse import bass_utils
from concourse.masks import make_identity

BF16 = ml_dtypes.bfloat16
DT = mybir.dt

# ---------------- problem config (hardcoded per contract) ----------------
N, E, F = 100000, 1600000, 256
H1, C1 = 8, 8          # layer1 heads x channels (concat -> 64)
C2 = 40                # layer2 single head, 40 classes
NEG = 0.2
NCORES = 8
NPC = N // NCORES      # 12500
BLK_W = 16             # nodes per window
TILE_E = 128           # edge slots per tile (= 1 window)
WPB = 8                # windows per block -> 128 positions
PAIR_W = 32            # window-pair positions (matmul M)
ROW1 = 72              # table1 row: [h1(64) | asrc(8)]
ROW2 = 41              # table2 row: [h2(40) | asrc2(1)]
OOB = 1 << 20   # out-of-bounds row sentinel; small enough that OOB*row_bytes
                # stays well inside int32 (descriptor index math wraps at 2^31)

_f32 = np.float32


# =================== host-side graph preprocessing ===================

def _pack_windows(deg):
    """FFD-pack nodes into windows (<=BLK_W nodes, <=TILE_E edges)."""
    windows = []
    open_bins = []  # (edges, idx)
    for n in np.argsort(-deg, kind="stable"):
        dn = int(deg[n])
        placed = False
        for bi, (e0, idx) in enumerate(open_bins):
            if e0 + dn <= TILE_E and len(windows[idx]) < BLK_W:
                windows[idx].append(int(n))
                if e0 + dn >= TILE_E or len(windows[idx]) >= BLK_W:
                    open_bins.pop(bi)
                else:
                    open_bins[bi] = (e0 + dn, idx)
                placed = True
                break
        if not placed:
            windows.append([int(n)])
            if dn < TILE_E:
                open_bins.append((dn, len(windows) - 1))
        if len(open_bins) > 48:
            open_bins.sort()
            del open_bins[32:]
    return windows


def preprocess(edge_index, k_tiles):
    """Build per-core index tensors. Returns (meta, per_core list of dicts)."""
    loop = np.arange(N, dtype=np.int64)
    src = np.concatenate([edge_index[0].astype(np.int64), loop])
    dst = np.concatenate([edge_index[1].astype(np.int64), loop])
    NPCP = -(-NPC // TILE_E) * TILE_E  # padded rows per core (12544)

    cores = []
    for c in range(NCORES):
        lo = c * NPC
        m = (dst >= lo) & (dst < lo + NPC)
        s_c, d_c = src[m], dst[m] - lo
        order = np.argsort(d_c, kind="stable")
        s_c, d_c = s_c[order], d_c[order]
        deg = np.bincount(d_c, minlength=NPC)
        windows = _pack_windows(deg)
        cores.append(dict(s=s_c, deg=deg, windows=windows))

    gran = int(np.lcm(WPB * WPB, k_tiles))  # nblocks % WPB == 0 for scatters
    ntiles = max(len(c["windows"]) for c in cores)
    ntiles = -(-ntiles // gran) * gran
    nblocks = ntiles // WPB
    S = ntiles // k_tiles

    def rowmap(n):
        return (n // NPC) * NPCP + (n % NPC)

    n_ptile = NPCP // TILE_E
    per_core = []
    for c in cores:
        c["windows"] += [[] for _ in range(ntiles - len(c["windows"]))]
        starts = np.zeros(NPC + 1, np.int64)
        starts[1:] = np.cumsum(c["deg"])
        gsrc = np.zeros((ntiles, TILE_E), np.int64)   # table row (pad->0)
        dloc = np.full((ntiles, TILE_E), PAIR_W, np.int64)  # pair-rel pos
        posmap = np.full((nblocks, WPB * BLK_W), OOB, np.int64)
        pscat = np.full(NPCP, OOB, np.int64)          # node -> global position
        for t, wnodes in enumerate(c["windows"]):
            b, w = divmod(t, WPB)
            ptr = 0
            for r, n in enumerate(wnodes):
                posmap[b, w * BLK_W + r] = n
                pscat[n] = t * BLK_W + r
                a, z = starts[n], starts[n + 1]
                k = z - a
                gsrc[t, ptr:ptr + k] = rowmap(c["s"][a:z])
                dloc[t, ptr:ptr + k] = r + BLK_W * (w % 2)
                ptr += k
            assert ptr <= TILE_E
        # [S, 128, K] layouts (lane-major per supertile)
        def sk(arr, dtype):
            return np.ascontiguousarray(
                arr.reshape(S, k_tiles, TILE_E).transpose(0, 2, 1)).astype(dtype)
        Bg = nblocks // WPB  # blocks grouped by 8 (rowi loaded per group)
        rowi = posmap.reshape(Bg, WPB, WPB * BLK_W).transpose(0, 2, 1)
        per_core.append(dict(
            idx=sk(gsrc, np.int32),
            dloc=sk(dloc, BF16),
            rowi=np.ascontiguousarray(rowi).astype(np.int32),
            pscat=np.ascontiguousarray(
                pscat.reshape(n_ptile, TILE_E, 1)).astype(np.int32),
        ))
    meta = dict(S=S, ntiles=ntiles, nblocks=nblocks, NPCP=NPCP, K=k_tiles)
    return meta, per_core


def build_weight_inputs(W1, att_src1, att_dst1, bias1, W2, att_src2, att_dst2,
                        bias2):
    """Pure layout rearrangement of weights (no FP arithmetic)."""
    A1 = np.zeros((64, 16), _f32)
    for h in range(H1):
        A1[h * 8:(h + 1) * 8, h] = att_src1[h]
        A1[h * 8:(h + 1) * 8, 8 + h] = att_dst1[h]
    att2 = np.concatenate([att_src2.T, att_dst2.T], axis=1).astype(_f32)
    b1r = np.broadcast_to(bias1.astype(_f32), (128, 64)).copy()
    b2r = np.broadcast_to(bias2.astype(_f32), (128, C2)).copy()
    return dict(W1=W1.astype(_f32), A1=A1, W2=W2.astype(_f32), att2=att2,
                b1r=b1r, b2r=b2r)


# =================== device program ===================

def _brd(ap, pattern, off=0):
    """Manual AP: keep partition dim, explicit free-dim [step,count] pattern."""
    return bass.AP(ap.tensor, ap.offset + off, [ap.ap[0]] + pattern)


def build_program(meta, debug=False):
    S, K, NPCP = meta["S"], meta["K"], meta["NPCP"]
    NT = NPCP * NCORES
    n_ptile = NPCP // 128

    nc = bacc.Bacc("TRN2", target_bir_lowering=False, debug=debug,
                   enable_asserts=False, num_devices=NCORES)

    def din(name, shape, dt):
        return nc.dram_tensor(name, shape, dt, kind="ExternalInput").ap()

    x_sl = din("x_sl", [NPCP, F], DT.float32)
    W1 = din("W1", [F, 64], DT.float32)
    A1 = din("A1", [64, 16], DT.float32)
    W2 = din("W2", [64, C2], DT.float32)
    att2 = din("att2", [C2, 2], DT.float32)
    b1r = din("b1r", [128, 64], DT.float32)
    b2r = din("b2r", [128, C2], DT.float32)
    idx_d = din("idx", [S, 128, K], DT.int32)
    dloc_d = din("dloc", [S, 128, K], DT.bfloat16)
    Bg = meta["nblocks"] // WPB
    rowi_d = din("rowi", [Bg, 128, WPB], DT.int32)
    pscat_d = din("pscat", [n_ptile, 128, 1], DT.int32)

    out_d = nc.dram_tensor("out", [NPCP, C2], DT.float32,
                           kind="ExternalOutput").ap()

    NPOS = meta["ntiles"] * BLK_W
    t1loc = nc.dram_tensor("t1loc", [NPCP, ROW1], DT.bfloat16).ap()
    t1full = nc.dram_tensor("t1full", [NT, ROW1], DT.bfloat16).ap()
    t2loc = nc.dram_tensor("t2loc", [NPCP, ROW2], DT.bfloat16).ap()
    t2full = nc.dram_tensor("t2full", [NT, ROW2], DT.bfloat16).ap()
    ad1pos = nc.dram_tensor("ad1pos", [NPOS, 8], DT.bfloat16).ap()
    ad2pos = nc.dram_tensor("ad2pos", [NPOS, 1], DT.bfloat16).ap()

    groups = [list(range(NCORES))]

    with tile.TileContext(nc, num_cores=NCORES) as tc:
        from contextlib import ExitStack
        with ExitStack() as top:
            cpool = top.enter_context(tc.tile_pool(name="const", bufs=1))
            id_f = cpool.tile([128, 128], DT.float32)
            make_identity(nc, id_f[:])
            id_b = cpool.tile([128, 128], DT.bfloat16)
            nc.vector.tensor_copy(id_b[:], id_f[:])
            iota32 = cpool.tile([128, PAIR_W], DT.bfloat16)
            iota32_i = cpool.tile([128, PAIR_W], DT.int16)
            nc.gpsimd.iota(iota32_i[:], pattern=[[1, PAIR_W]], base=0,
                           channel_multiplier=0)
            nc.vector.tensor_copy(iota32[:], iota32_i[:])
            iota16 = cpool.tile([128, BLK_W], DT.bfloat16)
            nc.vector.tensor_copy(iota16[:], iota32_i[:, 0:BLK_W])
            b1sb = cpool.tile([128, 64], DT.float32)
            nc.sync.dma_start(b1sb[:], b1r)
            b2sb = cpool.tile([128, C2], DT.float32)
            nc.sync.dma_start(b2sb[:], b2r)

            # ---------- P0: weight prep ----------
            # rhs1[i]: [128, 80] = [W1 | W1@A1s | W1@A1d] (f-tile i), bf16
            rhs1 = [cpool.tile([128, 80], DT.bfloat16, tag=f"rhs1_{i}",
                               name=f"rhs1_{i}") for i in range(2)]
            # rhs2: [64, 42] = [W2 | W2@as2 | W2@ad2], bf16
            rhs2 = cpool.tile([64, 42], DT.bfloat16)
            with tc.tile_pool(name="p0", bufs=1) as p0, \
                 tc.tile_pool(name="p0ps", bufs=1, space="PSUM") as p0ps:
                w1sb = [p0.tile([128, 64], DT.float32, tag=f"w1_{i}",
                                name=f"w1_{i}") for i in range(2)]
                for i in range(2):
                    nc.sync.dma_start(w1sb[i][:], W1[128 * i:128 * (i + 1), :])
                a1sb = p0.tile([64, 16], DT.float32)
                nc.sync.dma_start(a1sb[:], A1)
                w2sb = p0.tile([64, C2], DT.float32)
                nc.sync.dma_start(w2sb[:], W2)
                at2sb = p0.tile([C2, 2], DT.float32)
                nc.sync.dma_start(at2sb[:], att2)
                for i in range(2):
                    tp = p0ps.tile([64, 128], DT.float32, tag="w1t_ps")
                    nc.tensor.transpose(tp[:], w1sb[i][:], id_f[:])
                    w1t = p0.tile([64, 128], DT.float32, tag="w1t")
                    nc.vector.tensor_copy(w1t[:], tp[:])
                    wa = p0ps.tile([128, 16], DT.float32, tag="w1a_ps")
                    nc.tensor.matmul(wa[:], lhsT=w1t[:], rhs=a1sb[:],
                                     start=True, stop=True)
                    nc.vector.tensor_copy(rhs1[i][:, 0:64], w1sb[i][:])
                    nc.vector.tensor_copy(rhs1[i][:, 64:80], wa[:])
                tp2 = p0ps.tile([C2, 64], DT.float32, tag="w2t_ps")
                nc.tensor.transpose(tp2[:], w2sb[:], id_f[:64, :64])
                w2t = p0.tile([C2, 64], DT.float32)
                nc.vector.tensor_copy(w2t[:], tp2[:])
                wa2 = p0ps.tile([64, 2], DT.float32, tag="w2a_ps")
                nc.tensor.matmul(wa2[:], lhsT=w2t[:], rhs=at2sb[:],
                                 start=True, stop=True)
                nc.vector.tensor_copy(rhs2[:, 0:C2], w2sb[:])
                nc.vector.tensor_copy(rhs2[:, C2:C2 + 2], wa2[:])

            # ---------- P0.5: zero-init position-ordered adst tables ----------
            npos_f1 = NPOS * 8 // 128
            npos_f2 = NPOS // 128
            with tc.tile_pool(name="pz", bufs=1) as pz:
                zt = pz.tile([128, npos_f1], DT.bfloat16)
                nc.vector.memset(zt[:], 0.0)
                nc.sync.dma_start(
                    bass.AP(ad1pos.tensor, 0, [[npos_f1, 128], [1, npos_f1]]),
                    zt[:])
                nc.sync.dma_start(
                    bass.AP(ad2pos.tensor, 0, [[npos_f2, 128], [1, npos_f2]]),
                    zt[:, 0:npos_f2])

            # ---------- P1: dense layer-1 table ----------
            with tc.tile_pool(name="p1", bufs=3) as p1, \
                 tc.tile_pool(name="p1ps", bufs=2, space="PSUM") as p1ps:
                for it in range(n_ptile):
                    xt = p1.tile([128, F], DT.float32, tag="x")
                    nc.sync.dma_start(xt[:], x_sl[128 * it:128 * (it + 1), :])
                    xb = p1.tile([128, F], DT.bfloat16, tag="xb")
                    nc.vector.tensor_copy(xb[:], xt[:])
                    xT = p1.tile([128, F], DT.bfloat16, tag="xT")
                    ps1 = p1ps.tile([128, 80], DT.float32, tag="ps1")
                    for i in range(2):
                        tp = p1ps.tile([128, 128], DT.bfloat16, tag="xt_ps")
                        nc.tensor.transpose(
                            tp[:], xb[:, 128 * i:128 * (i + 1)], id_b[:])
                        nc.scalar.copy(xT[:, 128 * i:128 * (i + 1)], tp[:])
                    for i in range(2):
                        nc.tensor.matmul(
                            ps1[:], lhsT=xT[:, 128 * i:128 * (i + 1)],
                            rhs=rhs1[i][:], start=(i == 0), stop=(i == 1))
                    st = p1.tile([128, ROW1], DT.bfloat16, tag="st1")
                    nc.scalar.copy(st[:], ps1[:, 0:ROW1])
                    sta = p1.tile([128, 8], DT.bfloat16, tag="sta")
                    nc.scalar.copy(sta[:], ps1[:, 72:80])
                    nc.sync.dma_start(t1loc[128 * it:128 * (it + 1), :], st[:])
                    rwp = p1.tile([128, 1], DT.int32, tag="rwp")
                    nc.sync.dma_start(rwp[:], pscat_d[it])
                    nc.gpsimd.indirect_dma_start(
                        out=ad1pos,
                        out_offset=bass.IndirectOffsetOnAxis(ap=rwp[:], axis=0),
                        in_=sta[:], in_offset=None,
                        bounds_check=NPOS - 1, oob_is_err=False)

            # ---------- P2: AllGather table1 ----------
            nc.gpsimd.collective_compute(
                "AllGather", mybir.AluOpType.bypass, replica_groups=groups,
                ins=[t1loc.opt()], outs=[t1full.opt()])

            # ---------- P3: edge pass layer 1 ----------
            edge_pass(nc, tc, meta, 1, idx_d, dloc_d, rowi_d,
                      t1full, ad1pos, t2loc, ad2pos, None, iota16, iota32,
                      id_b, b1sb, rhs2)

            # ---------- P4: AllGather table2 ----------
            nc.gpsimd.collective_compute(
                "AllGather", mybir.AluOpType.bypass, replica_groups=groups,
                ins=[t2loc.opt()], outs=[t2full.opt()])

            # ---------- P5: edge pass layer 2 ----------
            edge_pass(nc, tc, meta, 2, idx_d, dloc_d, rowi_d,
                      t2full, ad2pos, None, None, out_d, iota16, iota32,
                      id_b, b2sb, None)

    nc.compile()
    return nc


def edge_pass(nc, tc, meta, layer, idx_d, dloc_d, rowi_d, tfull,
              adpos, t2loc, ad2pos, out_d, iota16, iota32, id_b, bias_sb,
              rhs2):
    S, K = meta["S"], meta["K"]
    row = ROW1 if layer == 1 else ROW2
    nh = H1 if layer == 1 else 1          # heads
    nch = 64 if layer == 1 else C2        # message channels
    nw = nch + nh                         # rhs width: [msgs*w | w]
    wb = BLK_W * nh                       # adst window elements
    from contextlib import ExitStack
    with ExitStack() as ctx:
        pm = ctx.enter_context(tc.tile_pool(name=f"e{layer}m", bufs=2))
        pg = ctx.enter_context(tc.tile_pool(name=f"e{layer}g", bufs=3))
        pw = ctx.enter_context(tc.tile_pool(name=f"e{layer}w", bufs=2))
        pb = ctx.enter_context(tc.tile_pool(name=f"e{layer}b", bufs=2))
        ps_blk = ctx.enter_context(
            tc.tile_pool(name=f"e{layer}ps", bufs=2, space="PSUM"))
        ps_b2 = ctx.enter_context(
            tc.tile_pool(name=f"e{layer}p2", bufs=2, space="PSUM"))
        state = {}
        for s in range(S):
            idx = pm.tile([128, K], DT.int32, tag="idx")
            nc.sync.dma_start(idx[:], idx_d[s])
            dl = pm.tile([128, K], DT.bfloat16, tag="dl")
            nc.sync.dma_start(dl[:], dloc_d[s])
            adb = pm.tile([128, K * wb], DT.bfloat16, tag="adb")
            nc.sync.dma_start(
                adb[:], bass.AP(adpos.tensor, s * K * wb,
                                [[0, 128], [1, K * wb]]))

            hs = pg.tile([128, K * row], DT.bfloat16, tag="hs")
            for j in range(K):
                nc.gpsimd.indirect_dma_start(
                    out=hs[:, row * j:row * (j + 1)], out_offset=None,
                    in_=tfull,
                    in_offset=bass.IndirectOffsetOnAxis(
                        ap=idx[:, j:j + 1], axis=0))

            dlw = pw.tile([128, K], DT.bfloat16, tag="dlw")
            nc.vector.tensor_scalar(out=dlw[:], in0=dl[:], scalar1=16.0,
                                    scalar2=-16.0,
                                    op0=mybir.AluOpType.is_ge,
                                    op1=mybir.AluOpType.mult)
            nc.vector.tensor_tensor(out=dlw[:], in0=dl[:], in1=dlw[:],
                                    op=mybir.AluOpType.add)
            s0w = pw.tile([128, K * BLK_W], DT.bfloat16, tag="s0w")
            nc.vector.tensor_tensor(
                out=_brd(s0w[:], [[BLK_W, K], [1, BLK_W]]),
                in0=_brd(iota16[:], [[0, K], [1, BLK_W]]),
                in1=_brd(dlw[:], [[1, K], [0, BLK_W]]),
                op=mybir.AluOpType.is_equal)
            spos = pw.tile([128, K * PAIR_W], DT.bfloat16, tag="spos")
            nc.vector.tensor_tensor(
                out=_brd(spos[:], [[PAIR_W, K], [1, PAIR_W]]),
                in0=_brd(iota32[:], [[0, K], [1, PAIR_W]]),
                in1=_brd(dl[:], [[1, K], [0, PAIR_W]]),
                op=mybir.AluOpType.is_equal)
            # a_dst expansion: m[(p), (j,h,r)] = onehot * adst_window
            m = pw.tile([128, K * wb], DT.bfloat16, tag="m")
            nc.vector.tensor_tensor(
                out=_brd(m[:], [[wb, K], [BLK_W, nh], [1, BLK_W]]),
                in0=_brd(s0w[:], [[BLK_W, K], [0, nh], [1, BLK_W]]),
                in1=_brd(adb[:], [[wb, K], [1, nh], [nh, BLK_W]]),
                op=mybir.AluOpType.mult)
            ea = pw.tile([128, K * nh], DT.float32, tag="ea")
            nc.vector.tensor_reduce(
                _brd(ea[:], [[nh, K], [1, nh]]),
                _brd(m[:], [[wb, K], [BLK_W, nh], [1, BLK_W]]),
                axis=mybir.AxisListType.X, op=mybir.AluOpType.add)
            e = pw.tile([128, K * nh], DT.float32, tag="e")
            nc.vector.tensor_tensor(
                out=_brd(e[:], [[nh, K], [1, nh]]),
                in0=_brd(hs[:], [[row, K], [1, nh]], off=nch),
                in1=_brd(ea[:], [[nh, K], [1, nh]]),
                op=mybir.AluOpType.add)
            tmp = pw.tile([128, K * nh], DT.float32, tag="etmp")
            nc.vector.tensor_scalar_mul(tmp[:], e[:], NEG)
            nc.vector.tensor_tensor(out=e[:], in0=e[:], in1=tmp[:],
                                    op=mybir.AluOpType.max)
            w = pw.tile([128, K * nh], DT.bfloat16, tag="w")
            nc.scalar.activation(w[:], e[:], mybir.ActivationFunctionType.Exp)
            hw = pw.tile([128, K * nw], DT.bfloat16, tag="hw")
            if layer == 1:
                nc.vector.tensor_tensor(
                    out=_brd(hw[:], [[nw, K], [8, 8], [1, 8]]),
                    in0=_brd(hs[:], [[row, K], [8, 8], [1, 8]]),
                    in1=_brd(w[:], [[nh, K], [1, 8], [0, 8]]),
                    op=mybir.AluOpType.mult)
                nc.vector.tensor_copy(
                    _brd(hw[:], [[nw, K], [1, 8]], off=64), w[:])
            else:
                nc.vector.tensor_tensor(
                    out=_brd(hw[:], [[nw, K], [1, C2]]),
                    in0=_brd(hs[:], [[row, K], [1, C2]]),
                    in1=_brd(w[:], [[1, K], [0, C2]]),
                    op=mybir.AluOpType.mult)
                nc.vector.tensor_copy(
                    _brd(hw[:], [[nw, K], [1, 1]], off=C2), w[:])

            for j in range(K):
                t = s * K + j
                wi = t % WPB
                p = wi // 2
                if wi == 0:
                    blk = ps_blk.tile([128, nw], DT.float32, tag="blk")
                nc.tensor.matmul(
                    blk[32 * p:32 * (p + 1), :],
                    lhsT=spos[:, PAIR_W * j:PAIR_W * (j + 1)],
                    rhs=hw[:, nw * j:nw * (j + 1)],
                    start=(wi % 2 == 0), stop=(wi % 2 == 1),
                    tile_position=(0, 32 * p), skip_group_check=True)
                if wi == WPB - 1:
                    b = t // WPB
                    if layer == 1:
                        _post1(nc, meta, b, blk, pb, ps_b2, rowi_d, t2loc,
                               ad2pos, id_b, bias_sb, rhs2, state)
                    else:
                        _post2(nc, meta, b, blk, pb, rowi_d, out_d,
                               bias_sb, state)


def _load_rowi(nc, pb, rowi_d, b, state, tag):
    bg, bi = divmod(b, WPB)
    if bi == 0:
        rw = pb.tile([128, WPB], DT.int32, tag=tag, name=tag)
        nc.sync.dma_start(rw[:], rowi_d[bg])
        state[tag] = rw
    return state[tag][:, b % WPB:b % WPB + 1]


def _post1(nc, meta, b, blk, pb, ps_b2, rowi_d, t2loc, ad2pos, id_b, b1sb,
           rhs2, state):
    """Finalize one 128-position block of layer 1, emit table-2 rows."""
    den = pb.tile([128, 8], DT.float32, tag="den")
    nc.vector.tensor_scalar_max(den[:], blk[:, 64:72], 1e-30)
    rec = pb.tile([128, 8], DT.float32, tag="rec")
    nc.vector.reciprocal(rec[:], den[:])
    hin = pb.tile([128, 64], DT.float32, tag="hin")
    for h in range(H1):
        nc.vector.tensor_scalar(
            out=hin[:, 8 * h:8 * (h + 1)], in0=blk[:, 8 * h:8 * (h + 1)],
            scalar1=rec[:, h:h + 1], scalar2=None, op0=mybir.AluOpType.mult)
    nc.vector.tensor_tensor(out=hin[:], in0=hin[:], in1=b1sb[:],
                            op=mybir.AluOpType.add)
    # ELU = max(x,0) + exp(min(x,0)) - 1
    emn = pb.tile([128, 64], DT.float32, tag="emn")
    nc.vector.tensor_scalar_min(emn[:], hin[:], 0.0)
    nc.scalar.activation(emn[:], emn[:], mybir.ActivationFunctionType.Exp)
    nc.vector.tensor_scalar_max(hin[:], hin[:], 0.0)
    nc.vector.tensor_tensor(out=hin[:], in0=hin[:], in1=emn[:],
                            op=mybir.AluOpType.add)
    helu = pb.tile([128, 64], DT.bfloat16, tag="helu")
    nc.vector.tensor_scalar_add(helu[:], hin[:], -1.0)
    htp = ps_b2.tile([64, 128], DT.bfloat16, tag="htp")
    nc.tensor.transpose(htp[:], helu[:], id_b[:])
    hts = pb.tile([64, 128], DT.bfloat16, tag="hts")
    nc.scalar.copy(hts[:], htp[:])
    h2ps = ps_b2.tile([128, 42], DT.float32, tag="h2ps")
    nc.tensor.matmul(h2ps[:], lhsT=hts[:], rhs=rhs2[:], start=True, stop=True)
    st2 = pb.tile([128, ROW2], DT.bfloat16, tag="st2")
    nc.scalar.copy(st2[:], h2ps[:, 0:ROW2])
    sta = pb.tile([128, 1], DT.bfloat16, tag="sta2")
    nc.scalar.copy(sta[:], h2ps[:, 41:42])
    rw = _load_rowi(nc, pb, rowi_d, b, state, "rw1")
    nc.gpsimd.indirect_dma_start(
        out=t2loc, out_offset=bass.IndirectOffsetOnAxis(ap=rw, axis=0),
        in_=st2[:], in_offset=None,
        bounds_check=meta["NPCP"] - 1, oob_is_err=False)
    nc.sync.dma_start(ad2pos[128 * b:128 * (b + 1), :], sta[:])


def _post2(nc, meta, b, blk, pb, rowi_d, out_d, b2sb, state):
    den = pb.tile([128, 1], DT.float32, tag="den2")
    nc.vector.tensor_scalar_max(den[:], blk[:, C2:C2 + 1], 1e-30)
    rec = pb.tile([128, 1], DT.float32, tag="rec2")
    nc.vector.reciprocal(rec[:], den[:])
    o2 = pb.tile([128, C2], DT.float32, tag="o2")
    nc.vector.tensor_scalar(out=o2[:], in0=blk[:, 0:C2], scalar1=rec[:],
                            scalar2=None, op0=mybir.AluOpType.mult)
    nc.vector.tensor_tensor(out=o2[:], in0=o2[:], in1=b2sb[:],
                            op=mybir.AluOpType.add)
    mx = pb.tile([128, 1], DT.float32, tag="mx")
    nc.vector.tensor_reduce(mx[:], o2[:], axis=mybir.AxisListType.X,
                            op=mybir.AluOpType.max)
    z = pb.tile([128, C2], DT.float32, tag="z")
    nc.vector.tensor_scalar(out=z[:], in0=o2[:], scalar1=mx[:], scalar2=None,
                            op0=mybir.AluOpType.subtract)
    ez = pb.tile([128, C2], DT.float32, tag="ez")
    se = pb.tile([128, 1], DT.float32, tag="se")
    nc.scalar.activation(ez[:], z[:], mybir.ActivationFunctionType.Exp,
                         accum_out=se[:])
    lse = pb.tile([128, 1], DT.float32, tag="lse")
    nc.scalar.activation(lse[:], se[:], mybir.ActivationFunctionType.Ln)
    zo = pb.tile([128, C2], DT.float32, tag="zo")
    nc.vector.tensor_scalar(out=zo[:], in0=z[:], scalar1=lse[:], scalar2=None,
                            op0=mybir.AluOpType.subtract)
    rw = _load_rowi(nc, pb, rowi_d, b, state, "rw2")
    nc.gpsimd.indirect_dma_start(
        out=out_d, out_offset=bass.IndirectOffsetOnAxis(ap=rw, axis=0),
        in_=zo[:], in_offset=None,
        bounds_check=meta["NPCP"] - 1, oob_is_err=False)


# =================== SPMD runner (bass2jax-based, with timing) ===================

def _run_spmd(nc, in_maps, n_timing_iters=0):
    """Execute the program on NCORES neuron devices via PJRT (axon).

    Modeled on bass2jax.run_bass_via_pjrt's multi-core branch, but jits once,
    keeps inputs resident on device, and optionally times repeated runs.
    Returns (per_core_results, wall_times_s).
    """
    import jax
    from jax.sharding import Mesh, PartitionSpec
    from jax.experimental.shard_map import shard_map
    from concourse import bass2jax
    from concourse.bass2jax import _bass_exec_p, partition_id_tensor
    import time

    bass2jax.install_neuronx_cc_hook()
    assert nc.dbg_addr is None or not nc.dbg_callbacks

    in_names, out_names, out_avals, zero_outs = [], [], [], []
    partition_name = (nc.partition_id_tensor.name
                      if nc.partition_id_tensor else None)
    for alloc in nc.m.functions[0].allocations:
        if not isinstance(alloc, mybir.MemoryLocationSet):
            continue
        name = alloc.memorylocations[0].name
        if alloc.kind == "ExternalInput":
            if name != partition_name:
                in_names.append(name)
        elif alloc.kind == "ExternalOutput":
            out_names.append(name)
            shape = tuple(alloc.tensor_shape)
            dtype = mybir.dt.np(alloc.dtype)
            out_avals.append(jax.core.ShapedArray(shape, dtype))
            zero_outs.append(np.zeros(shape, dtype))
    n_params = len(in_names)
    all_in_names = in_names + out_names + (
        [partition_name] if partition_name else [])

    def _body(*args):
        operands = list(args)
        if partition_name is not None:
            operands.append(partition_id_tensor())
        return tuple(_bass_exec_p.bind(
            *operands,
            out_avals=tuple(out_avals),
            in_names=tuple(all_in_names),
            out_names=tuple(out_names),
            lowering_input_output_aliases=(),
            sim_require_finite=True,
            sim_require_nnan=True,
            nc=nc,
        ))

    devices = jax.devices()[:NCORES]
    mesh = Mesh(np.asarray(devices), ("core",))
    nin = n_params + len(out_names)
    fn = jax.jit(shard_map(_body, mesh=mesh,
                           in_specs=(PartitionSpec("core"),) * nin,
                           out_specs=(PartitionSpec("core"),) * len(out_names),
                           check_rep=False),
                 keep_unused=True)
    sh = jax.sharding.NamedSharding(mesh, PartitionSpec("core"))
    concat_in = [
        jax.device_put(np.concatenate(
            [np.asarray(in_maps[c][name]) for c in range(NCORES)], axis=0), sh)
        for name in in_names
    ]
    concat_zeros = [
        jax.device_put(np.zeros((NCORES * z.shape[0], *z.shape[1:]), z.dtype),
                       sh) for z in zero_outs
    ]
    out_arrs = jax.block_until_ready(fn(*concat_in, *concat_zeros))
    times = []
    for _ in range(n_timing_iters):
        t0 = time.perf_counter()
        r = jax.block_until_ready(fn(*concat_in, *concat_zeros))
        times.append(time.perf_counter() - t0)
        del r
    results = [
        {name: np.asarray(out_arrs[i]).reshape(NCORES, *out_avals[i].shape)[c]
         for i, name in enumerate(out_names)}
        for c in range(NCORES)
    ]
    return results, times


# =================== top-level entry ===================

def kernel(**inputs):
    K_TILES = 64
    edge_index = np.asarray(inputs["edge_index"])
    meta, per_core = preprocess(edge_index, K_TILES)
    wts = build_weight_inputs(
        np.asarray(inputs["W1"]), np.asarray(inputs["att_src1"]),
        np.asarray(inputs["att_dst1"]), np.asarray(inputs["bias1"]),
        np.asarray(inputs["W2"]), np.asarray(inputs["att_src2"]),
        np.asarray(inputs["att_dst2"]), np.asarray(inputs["bias2"]))
    x = np.asarray(inputs["x"], _f32)
    NPCP = meta["NPCP"]
    in_maps = []
    for c in range(NCORES):
        xs = np.zeros((NPCP, F), _f32)
        xs[:NPC] = x[c * NPC:(c + 1) * NPC]
        in_maps.append(dict(
            x_sl=xs, W1=wts["W1"], A1=wts["A1"], W2=wts["W2"],
            att2=wts["att2"], b1r=wts["b1r"], b2r=wts["b2r"],
            idx=per_core[c]["idx"], dloc=per_core[c]["dloc"],
            rowi=per_core[c]["rowi"], pscat=per_core[c]["pscat"]))
    nc = build_program(meta)
    n_iters = int(os.environ.get("GAT_BENCH_ITERS", "0"))
    results, times = _run_spmd(nc, in_maps, n_timing_iters=n_iters)
    global LAST_TIMES
    LAST_TIMES = times
    out = np.zeros((N, C2), _f32)
    res = type("R", (), {"results": results})()
    for c in range(NCORES):
        out[c * NPC:(c + 1) * NPC] = res.results[c]["out"][:NPC]
    return out


# revision 31
# speedup vs baseline: 15.3672x; 15.3672x over previous
"""GAT (2-layer, PyG-style) distributed Bass kernel for 8 TRN2 NeuronCores.

Strategy (sharding_hint: 1D node partition by dst):
  - core c owns dst nodes [c*NPC, (c+1)*NPC)
  - dense phase: each core computes table1 = [h1(64) | a_src1(8)] rows for its
    node slice (x_slice @ [W1 | W1@A1s | W1@A1d]), AllGather -> full table in
    every core's DRAM.
  - edge phase: host packs edges into "windows" (<=16 dst nodes, <=128 edges;
    1 window == 1 tile of 128 edge slots; 8 windows = 1 block of 128 node
    positions).  Per tile: indirect-DMA gather of source rows + a_dst rows,
    softmax weights w = exp(leakyrelu(a_src+a_dst)) on-chip (no max
    subtraction; |logit| < ~3 verified), then ONE TensorE matmul per tile:
      blk[pos, 0:64 | 64:72] += S_pos^T @ [ h1*w_head | w ]
    where S_pos is the one-hot (edge -> window-pair position) matrix as lhsT
    (col-tiled, tile_position=(0,32p)).  Denominators land in cols 64:72.
  - block post: normalize per head, +bias, ELU, transpose once, W2aug matmul
    -> layer-2 table rows [h2(40) | a_src2(1)] + a_dst2, scattered to DRAM by
    node index (OOB rows skipped); AllGather #2; same edge pass for layer 2;
    log_softmax; scatter rows to the output.
All floating-point math runs on-device.  Host work is integer graph
preprocessing (sort/pack/index-building) and weight layout rearrangement.
"""
import os
import sys
import numpy as np

try:
    import concourse.bass as bass
except ImportError:  # pragma: no cover
    for p in ("/opt/trn_rl_repo", "/root/.axon_site/_ro/trn_rl_repo"):
        if os.path.isdir(p) and p not in sys.path:
            sys.path.insert(0, p)
    import concourse.bass as bass

import ml_dtypes
import concourse.mybir as mybir
import concourse.tile as tile
import concourse.bacc as bacc
from concourse import bass_utils
from concourse.masks import make_identity

BF16 = ml_dtypes.bfloat16
DT = mybir.dt

# ---------------- problem config (hardcoded per contract) ----------------
N, E, F = 100000, 1600000, 256
H1, C1 = 8, 8          # layer1 heads x channels (concat -> 64)
C2 = 40                # layer2 single head, 40 classes
NEG = 0.2
NCORES = 8
NPC = N // NCORES      # 12500
BLK_W = 16             # nodes per window
TILE_E = 128           # edge slots per tile (= 1 window)
WPB = 8                # windows per block -> 128 positions
PAIR_W = 32            # window-pair positions (matmul M)
ROW1 = 72              # table1 row: [h1(64) | asrc(8)]
ROW2 = 41              # table2 row: [h2(40) | asrc2(1)]
OOB = 1 << 20   # out-of-bounds row sentinel; small enough that OOB*row_bytes
                # stays well inside int32 (descriptor index math wraps at 2^31)

_f32 = np.float32


# =================== host-side graph preprocessing ===================

def _pack_windows(deg):
    """FFD-pack nodes into windows (<=BLK_W nodes, <=TILE_E edges)."""
    windows = []
    open_bins = []  # (edges, idx)
    for n in np.argsort(-deg, kind="stable"):
        dn = int(deg[n])
        placed = False
        for bi, (e0, idx) in enumerate(open_bins):
            if e0 + dn <= TILE_E and len(windows[idx]) < BLK_W:
                windows[idx].append(int(n))
                if e0 + dn >= TILE_E or len(windows[idx]) >= BLK_W:
                    open_bins.pop(bi)
                else:
                    open_bins[bi] = (e0 + dn, idx)
                placed = True
                break
        if not placed:
            windows.append([int(n)])
            if dn < TILE_E:
                open_bins.append((dn, len(windows) - 1))
        if len(open_bins) > 48:
            open_bins.sort()
            del open_bins[32:]
    return windows


def preprocess(edge_index, k_tiles):
    """Build per-core index tensors. Returns (meta, per_core list of dicts)."""
    loop = np.arange(N, dtype=np.int64)
    src = np.concatenate([edge_index[0].astype(np.int64), loop])
    dst = np.concatenate([edge_index[1].astype(np.int64), loop])
    NPCP = -(-NPC // TILE_E) * TILE_E  # padded rows per core (12544)

    cores = []
    for c in range(NCORES):
        lo = c * NPC
        m = (dst >= lo) & (dst < lo + NPC)
        s_c, d_c = src[m], dst[m] - lo
        order = np.argsort(d_c, kind="stable")
        s_c, d_c = s_c[order], d_c[order]
        deg = np.bincount(d_c, minlength=NPC)
        windows = _pack_windows(deg)
        cores.append(dict(s=s_c, deg=deg, windows=windows))

    gran = int(np.lcm(WPB * WPB, k_tiles))  # nblocks % WPB == 0 for scatters
    ntiles = max(len(c["windows"]) for c in cores)
    ntiles = -(-ntiles // gran) * gran
    nblocks = ntiles // WPB
    S = ntiles // k_tiles

    def rowmap(n):
        return (n // NPC) * NPCP + (n % NPC)

    n_ptile = NPCP // TILE_E
    per_core = []
    for c in cores:
        c["windows"] += [[] for _ in range(ntiles - len(c["windows"]))]
        starts = np.zeros(NPC + 1, np.int64)
        starts[1:] = np.cumsum(c["deg"])
        gsrc = np.zeros((ntiles, TILE_E), np.int64)   # table row (pad->0)
        dloc = np.full((ntiles, TILE_E), PAIR_W, np.int64)  # pair-rel pos
        posmap = np.full((nblocks, WPB * BLK_W), OOB, np.int64)
        pscat = np.full(NPCP, OOB, np.int64)          # node -> global position
        for t, wnodes in enumerate(c["windows"]):
            b, w = divmod(t, WPB)
            ptr = 0
            for r, n in enumerate(wnodes):
                posmap[b, w * BLK_W + r] = n
                pscat[n] = t * BLK_W + r
                a, z = starts[n], starts[n + 1]
                k = z - a
                gsrc[t, ptr:ptr + k] = rowmap(c["s"][a:z])
                dloc[t, ptr:ptr + k] = r + BLK_W * (w % 2)
                ptr += k
            assert ptr <= TILE_E
        # [S, 128, K] layouts (lane-major per supertile)
        def sk(arr, dtype):
            return np.ascontiguousarray(
                arr.reshape(S, k_tiles, TILE_E).transpose(0, 2, 1)).astype(dtype)
        Bg = nblocks // WPB  # blocks grouped by 8 (rowi loaded per group)
        rowi = posmap.reshape(Bg, WPB, WPB * BLK_W).transpose(0, 2, 1)
        per_core.append(dict(
            idx=sk(gsrc, np.int32),
            dloc=sk(dloc, BF16),
            rowi=np.ascontiguousarray(rowi).astype(np.int32),
            pscat=np.ascontiguousarray(
                pscat.reshape(n_ptile, TILE_E, 1)).astype(np.int32),
            posmap_flat=posmap.reshape(-1).copy(),
        ))
    meta = dict(S=S, ntiles=ntiles, nblocks=nblocks, NPCP=NPCP, K=k_tiles)
    return meta, per_core


def build_weight_inputs(W1, att_src1, att_dst1, bias1, W2, att_src2, att_dst2,
                        bias2):
    """Pure layout rearrangement of weights (no FP arithmetic)."""
    A1 = np.zeros((64, 16), _f32)
    for h in range(H1):
        A1[h * 8:(h + 1) * 8, h] = att_src1[h]
        A1[h * 8:(h + 1) * 8, 8 + h] = att_dst1[h]
    att2 = np.concatenate([att_src2.T, att_dst2.T], axis=1).astype(_f32)
    b1r = np.broadcast_to(bias1.astype(_f32), (128, 64)).copy()
    b2r = np.broadcast_to(bias2.astype(_f32), (128, C2)).copy()
    return dict(W1=W1.astype(_f32), A1=A1, W2=W2.astype(_f32), att2=att2,
                b1r=b1r, b2r=b2r)


# =================== device program ===================

def _brd(ap, pattern, off=0):
    """Manual AP: keep partition dim, explicit free-dim [step,count] pattern."""
    return bass.AP(ap.tensor, ap.offset + off, [ap.ap[0]] + pattern)


def build_program(meta, debug=False, phases=(1, 2)):
    S, K, NPCP = meta["S"], meta["K"], meta["NPCP"]
    NT = NPCP * NCORES
    n_ptile = NPCP // 128

    nc = bacc.Bacc("TRN2", target_bir_lowering=False, debug=debug,
                   enable_asserts=False, num_devices=NCORES)

    def din(name, shape, dt):
        return nc.dram_tensor(name, shape, dt, kind="ExternalInput").ap()

    x_sl = din("x_sl", [NPCP, F], DT.float32)
    W1 = din("W1", [F, 64], DT.float32)
    A1 = din("A1", [64, 16], DT.float32)
    W2 = din("W2", [64, C2], DT.float32)
    att2 = din("att2", [C2, 2], DT.float32)
    b1r = din("b1r", [128, 64], DT.float32)
    b2r = din("b2r", [128, C2], DT.float32)
    idx_d = din("idx", [S, 128, K], DT.int32)
    dloc_d = din("dloc", [S, 128, K], DT.bfloat16)
    Bg = meta["nblocks"] // WPB
    rowi_d = din("rowi", [Bg, 128, WPB], DT.int32)
    pscat_d = din("pscat", [n_ptile, 128, 1], DT.int32)

    NPOS0 = meta["ntiles"] * BLK_W
    out_d = nc.dram_tensor("out", [NPOS0, C2], DT.float32,
                           kind="ExternalOutput").ap()

    NPOS = meta["ntiles"] * BLK_W
    t1loc = nc.dram_tensor("t1loc", [NPCP, ROW1], DT.bfloat16).ap()
    t1full = nc.dram_tensor("t1full", [NT, ROW1], DT.bfloat16,
                            addr_space="Shared").ap()
    t2loc = nc.dram_tensor("t2loc", [NPCP, ROW2], DT.bfloat16).ap()
    t2full = nc.dram_tensor("t2full", [NT, ROW2], DT.bfloat16,
                            addr_space="Shared").ap()
    ad1pos = nc.dram_tensor("ad1pos", [NPOS, 8], DT.bfloat16).ap()
    ad2pos = nc.dram_tensor("ad2pos", [NPOS, 1], DT.bfloat16).ap()

    groups = [list(range(NCORES))]

    with tile.TileContext(nc, num_cores=NCORES) as tc:
        from contextlib import ExitStack
        with ExitStack() as top:
            cpool = top.enter_context(tc.tile_pool(name="const", bufs=1))
            id_f = cpool.tile([128, 128], DT.float32)
            make_identity(nc, id_f[:])
            id_b = cpool.tile([128, 128], DT.bfloat16)
            nc.vector.tensor_copy(id_b[:], id_f[:])
            iota32 = cpool.tile([128, PAIR_W], DT.bfloat16)
            iota32_i = cpool.tile([128, PAIR_W], DT.int16)
            nc.gpsimd.iota(iota32_i[:], pattern=[[1, PAIR_W]], base=0,
                           channel_multiplier=0)
            nc.vector.tensor_copy(iota32[:], iota32_i[:])
            iota16 = cpool.tile([128, BLK_W], DT.bfloat16)
            nc.vector.tensor_copy(iota16[:], iota32_i[:, 0:BLK_W])
            b1sb = cpool.tile([128, 64], DT.float32)
            nc.sync.dma_start(b1sb[:], b1r)
            b2sb = cpool.tile([128, C2], DT.float32)
            nc.sync.dma_start(b2sb[:], b2r)

            # ---------- P0: weight prep ----------
            # rhs1[i]: [128, 80] = [W1 | W1@A1s | W1@A1d] (f-tile i), bf16
            rhs1 = [cpool.tile([128, 80], DT.bfloat16, tag=f"rhs1_{i}",
                               name=f"rhs1_{i}") for i in range(2)]
            # rhs2: [64, 42] = [W2 | W2@as2 | W2@ad2], bf16
            rhs2 = cpool.tile([64, 42], DT.bfloat16)
            with tc.tile_pool(name="p0", bufs=1) as p0, \
                 tc.tile_pool(name="p0ps", bufs=1, space="PSUM") as p0ps:
                w1sb = [p0.tile([128, 64], DT.float32, tag=f"w1_{i}",
                                name=f"w1_{i}") for i in range(2)]
                for i in range(2):
                    nc.sync.dma_start(w1sb[i][:], W1[128 * i:128 * (i + 1), :])
                a1sb = p0.tile([64, 16], DT.float32)
                nc.sync.dma_start(a1sb[:], A1)
                w2sb = p0.tile([64, C2], DT.float32)
                nc.sync.dma_start(w2sb[:], W2)
                at2sb = p0.tile([C2, 2], DT.float32)
                nc.sync.dma_start(at2sb[:], att2)
                for i in range(2):
                    tp = p0ps.tile([64, 128], DT.float32, tag="w1t_ps")
                    nc.tensor.transpose(tp[:], w1sb[i][:], id_f[:])
                    w1t = p0.tile([64, 128], DT.float32, tag="w1t")
                    nc.vector.tensor_copy(w1t[:], tp[:])
                    wa = p0ps.tile([128, 16], DT.float32, tag="w1a_ps")
                    nc.tensor.matmul(wa[:], lhsT=w1t[:], rhs=a1sb[:],
                                     start=True, stop=True)
                    nc.vector.tensor_copy(rhs1[i][:, 0:64], w1sb[i][:])
                    nc.vector.tensor_copy(rhs1[i][:, 64:80], wa[:])
                tp2 = p0ps.tile([C2, 64], DT.float32, tag="w2t_ps")
                nc.tensor.transpose(tp2[:], w2sb[:], id_f[:64, :64])
                w2t = p0.tile([C2, 64], DT.float32)
                nc.vector.tensor_copy(w2t[:], tp2[:])
                wa2 = p0ps.tile([64, 2], DT.float32, tag="w2a_ps")
                nc.tensor.matmul(wa2[:], lhsT=w2t[:], rhs=at2sb[:],
                                 start=True, stop=True)
                nc.vector.tensor_copy(rhs2[:, 0:C2], w2sb[:])
                nc.vector.tensor_copy(rhs2[:, C2:C2 + 2], wa2[:])

            # ---------- P0.5: zero-init position-ordered adst tables ----------
            npos_f1 = NPOS * 8 // 128
            npos_f2 = NPOS // 128
            with tc.tile_pool(name="pz", bufs=1) as pz:
                zt = pz.tile([128, npos_f1], DT.bfloat16)
                nc.vector.memset(zt[:], 0.0)
                nc.sync.dma_start(
                    bass.AP(ad1pos.tensor, 0, [[npos_f1, 128], [1, npos_f1]]),
                    zt[:])
                nc.sync.dma_start(
                    bass.AP(ad2pos.tensor, 0, [[npos_f2, 128], [1, npos_f2]]),
                    zt[:, 0:npos_f2])

            # ---------- P1: dense layer-1 table ----------
            with tc.tile_pool(name="p1", bufs=3) as p1, \
                 tc.tile_pool(name="p1ps", bufs=2, space="PSUM") as p1ps:
                for it in range(n_ptile):
                    xt = p1.tile([128, F], DT.float32, tag="x")
                    nc.sync.dma_start(xt[:], x_sl[128 * it:128 * (it + 1), :])
                    xb = p1.tile([128, F], DT.bfloat16, tag="xb")
                    nc.vector.tensor_copy(xb[:], xt[:])
                    xT = p1.tile([128, F], DT.bfloat16, tag="xT")
                    ps1 = p1ps.tile([128, 80], DT.float32, tag="ps1")
                    for i in range(2):
                        tp = p1ps.tile([128, 128], DT.bfloat16, tag="xt_ps")
                        nc.tensor.transpose(
                            tp[:], xb[:, 128 * i:128 * (i + 1)], id_b[:])
                        nc.scalar.copy(xT[:, 128 * i:128 * (i + 1)], tp[:])
                    for i in range(2):
                        nc.tensor.matmul(
                            ps1[:], lhsT=xT[:, 128 * i:128 * (i + 1)],
                            rhs=rhs1[i][:], start=(i == 0), stop=(i == 1))
                    st = p1.tile([128, ROW1], DT.bfloat16, tag="st1")
                    nc.scalar.copy(st[:], ps1[:, 0:ROW1])
                    sta = p1.tile([128, 8], DT.bfloat16, tag="sta")
                    nc.scalar.copy(sta[:], ps1[:, 72:80])
                    nc.sync.dma_start(t1loc[128 * it:128 * (it + 1), :], st[:])
                    rwp = p1.tile([128, 1], DT.int32, tag="rwp")
                    nc.sync.dma_start(rwp[:], pscat_d[it])
                    nc.gpsimd.indirect_dma_start(
                        out=ad1pos,
                        out_offset=bass.IndirectOffsetOnAxis(ap=rwp[:], axis=0),
                        in_=sta[:], in_offset=None,
                        bounds_check=NPOS - 1, oob_is_err=False)

            # ---------- P2: AllGather table1 ----------
            nc.gpsimd.collective_compute(
                "AllGather", mybir.AluOpType.bypass, replica_groups=groups,
                ins=[t1loc.opt()], outs=[t1full.opt()])

            # ---------- P3: edge pass layer 1 ----------
            if 1 in phases:
                edge_pass(nc, tc, meta, 1, idx_d, dloc_d, rowi_d,
                          t1full, ad1pos, t2loc, ad2pos, None, iota16,
                          iota32, id_b, b1sb, rhs2)

            # ---------- P4: AllGather table2 ----------
            nc.gpsimd.collective_compute(
                "AllGather", mybir.AluOpType.bypass, replica_groups=groups,
                ins=[t2loc.opt()], outs=[t2full.opt()])

            # ---------- P5: edge pass layer 2 ----------
            if 2 in phases:
                edge_pass(nc, tc, meta, 2, idx_d, dloc_d, rowi_d,
                          t2full, ad2pos, None, None, out_d, iota16,
                          iota32, id_b, b2sb, None)
            else:
                zo = cpool.tile([128, C2], DT.float32)
                nc.vector.memset(zo[:], 0.0)
                nc.sync.dma_start(out_d[0:128, :], zo[:])

    nc.compile()
    return nc


def edge_pass(nc, tc, meta, layer, idx_d, dloc_d, rowi_d, tfull,
              adpos, t2loc, ad2pos, out_d, iota16, iota32, id_b, bias_sb,
              rhs2):
    S, K = meta["S"], meta["K"]
    row = ROW1 if layer == 1 else ROW2
    nh = H1 if layer == 1 else 1          # heads
    nch = 64 if layer == 1 else C2        # message channels
    nw = nch + nh                         # rhs width: [msgs*w | w]
    wb = BLK_W * nh                       # adst window elements
    from contextlib import ExitStack
    with ExitStack() as ctx:
        pm = ctx.enter_context(tc.tile_pool(name=f"e{layer}m", bufs=2))
        pg = ctx.enter_context(tc.tile_pool(name=f"e{layer}g", bufs=3))
        pw = ctx.enter_context(tc.tile_pool(name=f"e{layer}w", bufs=3))
        pb = ctx.enter_context(tc.tile_pool(name=f"e{layer}b", bufs=2))
        ps_blk = ctx.enter_context(
            tc.tile_pool(name=f"e{layer}ps", bufs=3, space="PSUM"))
        ps_b2 = ctx.enter_context(
            tc.tile_pool(name=f"e{layer}p2", bufs=2, space="PSUM"))
        state = {}
        for s in range(S):
            idx = pm.tile([128, K], DT.int32, tag="idx")
            nc.sync.dma_start(idx[:], idx_d[s])
            dl = pm.tile([128, K], DT.bfloat16, tag="dl")
            nc.sync.dma_start(dl[:], dloc_d[s])
            adb = pm.tile([128, K * wb], DT.bfloat16, tag="adb")
            nc.sync.dma_start(
                adb[:], bass.AP(adpos.tensor, s * K * wb,
                                [[0, 128], [1, K * wb]]))

            hs = pg.tile([128, K * row], DT.bfloat16, tag="hs")
            for j in range(K):
                nc.gpsimd.indirect_dma_start(
                    out=hs[:, row * j:row * (j + 1)], out_offset=None,
                    in_=tfull,
                    in_offset=bass.IndirectOffsetOnAxis(
                        ap=idx[:, j:j + 1], axis=0))

            dlw = pw.tile([128, K], DT.bfloat16, tag="dlw")
            nc.vector.tensor_scalar(out=dlw[:], in0=dl[:], scalar1=16.0,
                                    scalar2=-16.0,
                                    op0=mybir.AluOpType.is_ge,
                                    op1=mybir.AluOpType.mult)
            nc.vector.tensor_tensor(out=dlw[:], in0=dl[:], in1=dlw[:],
                                    op=mybir.AluOpType.add)
            s0w = pw.tile([128, K * BLK_W], DT.bfloat16, tag="s0w")
            nc.vector.tensor_tensor(
                out=_brd(s0w[:], [[BLK_W, K], [1, BLK_W]]),
                in0=_brd(iota16[:], [[0, K], [1, BLK_W]]),
                in1=_brd(dlw[:], [[1, K], [0, BLK_W]]),
                op=mybir.AluOpType.is_equal)
            spos = pw.tile([128, K * PAIR_W], DT.bfloat16, tag="spos")
            nc.vector.tensor_tensor(
                out=_brd(spos[:], [[PAIR_W, K], [1, PAIR_W]]),
                in0=_brd(iota32[:], [[0, K], [1, PAIR_W]]),
                in1=_brd(dl[:], [[1, K], [0, PAIR_W]]),
                op=mybir.AluOpType.is_equal)
            # a_dst expansion: m[(p), (j,h,r)] = onehot * adst_window
            m = pw.tile([128, K * wb], DT.bfloat16, tag="m")
            nc.vector.tensor_tensor(
                out=_brd(m[:], [[wb, K], [BLK_W, nh], [1, BLK_W]]),
                in0=_brd(s0w[:], [[BLK_W, K], [0, nh], [1, BLK_W]]),
                in1=_brd(adb[:], [[wb, K], [1, nh], [nh, BLK_W]]),
                op=mybir.AluOpType.mult)
            ea = pw.tile([128, K * nh], DT.float32, tag="ea")
            nc.vector.tensor_reduce(
                _brd(ea[:], [[nh, K], [1, nh]]),
                _brd(m[:], [[wb, K], [BLK_W, nh], [1, BLK_W]]),
                axis=mybir.AxisListType.X, op=mybir.AluOpType.add)
            e = pw.tile([128, K * nh], DT.float32, tag="e")
            nc.vector.tensor_tensor(
                out=_brd(e[:], [[nh, K], [1, nh]]),
                in0=_brd(hs[:], [[row, K], [1, nh]], off=nch),
                in1=_brd(ea[:], [[nh, K], [1, nh]]),
                op=mybir.AluOpType.add)
            tmp = pw.tile([128, K * nh], DT.float32, tag="etmp")
            nc.vector.tensor_scalar_mul(tmp[:], e[:], NEG)
            nc.vector.tensor_tensor(out=e[:], in0=e[:], in1=tmp[:],
                                    op=mybir.AluOpType.max)
            w = pw.tile([128, K * nh], DT.bfloat16, tag="w")
            nc.scalar.activation(w[:], e[:], mybir.ActivationFunctionType.Exp)
            hw = pw.tile([128, K * nw], DT.bfloat16, tag="hw")
            if layer == 1:
                nc.vector.tensor_tensor(
                    out=_brd(hw[:], [[nw, K], [8, 8], [1, 8]]),
                    in0=_brd(hs[:], [[row, K], [8, 8], [1, 8]]),
                    in1=_brd(w[:], [[nh, K], [1, 8], [0, 8]]),
                    op=mybir.AluOpType.mult)
                nc.vector.tensor_copy(
                    _brd(hw[:], [[nw, K], [1, 8]], off=64), w[:])
            else:
                nc.vector.tensor_tensor(
                    out=_brd(hw[:], [[nw, K], [1, C2]]),
                    in0=_brd(hs[:], [[row, K], [1, C2]]),
                    in1=_brd(w[:], [[1, K], [0, C2]]),
                    op=mybir.AluOpType.mult)
                nc.vector.tensor_copy(
                    _brd(hw[:], [[nw, K], [1, 1]], off=C2), w[:])

            for j in range(K):
                t = s * K + j
                wi = t % WPB
                p = wi // 2
                if wi == 0:
                    blk = ps_blk.tile([128, nw], DT.float32, tag="blk")
                nc.tensor.matmul(
                    blk[32 * p:32 * (p + 1), :],
                    lhsT=spos[:, PAIR_W * j:PAIR_W * (j + 1)],
                    rhs=hw[:, nw * j:nw * (j + 1)],
                    start=(wi % 2 == 0), stop=(wi % 2 == 1),
                    tile_position=(0, 32 * p), skip_group_check=True)
                if wi == WPB - 1:
                    b = t // WPB
                    if layer == 1:
                        _post1(nc, meta, b, blk, pb, ps_b2, rowi_d, t2loc,
                               ad2pos, id_b, bias_sb, rhs2, state)
                    else:
                        _post2(nc, meta, b, blk, pb, rowi_d, out_d,
                               bias_sb, state)


def _load_rowi(nc, pb, rowi_d, b, state, tag):
    bg, bi = divmod(b, WPB)
    if bi == 0:
        rw = pb.tile([128, WPB], DT.int32, tag=tag, name=tag)
        nc.sync.dma_start(rw[:], rowi_d[bg])
        state[tag] = rw
    return state[tag][:, b % WPB:b % WPB + 1]


def _post1(nc, meta, b, blk, pb, ps_b2, rowi_d, t2loc, ad2pos, id_b, b1sb,
           rhs2, state):
    """Finalize one 128-position block of layer 1, emit table-2 rows."""
    den = pb.tile([128, 8], DT.float32, tag="den")
    nc.vector.tensor_scalar_max(den[:], blk[:, 64:72], 1e-30)
    rec = pb.tile([128, 8], DT.float32, tag="rec")
    nc.vector.reciprocal(rec[:], den[:])
    hin = pb.tile([128, 64], DT.float32, tag="hin")
    for h in range(H1):
        nc.vector.tensor_scalar(
            out=hin[:, 8 * h:8 * (h + 1)], in0=blk[:, 8 * h:8 * (h + 1)],
            scalar1=rec[:, h:h + 1], scalar2=None, op0=mybir.AluOpType.mult)
    nc.vector.tensor_tensor(out=hin[:], in0=hin[:], in1=b1sb[:],
                            op=mybir.AluOpType.add)
    # ELU = max(x,0) + exp(min(x,0)) - 1
    emn = pb.tile([128, 64], DT.float32, tag="emn")
    nc.vector.tensor_scalar_min(emn[:], hin[:], 0.0)
    nc.scalar.activation(emn[:], emn[:], mybir.ActivationFunctionType.Exp)
    nc.vector.tensor_scalar_max(hin[:], hin[:], 0.0)
    nc.vector.tensor_tensor(out=hin[:], in0=hin[:], in1=emn[:],
                            op=mybir.AluOpType.add)
    helu = pb.tile([128, 64], DT.bfloat16, tag="helu")
    nc.vector.tensor_scalar_add(helu[:], hin[:], -1.0)
    htp = ps_b2.tile([64, 128], DT.bfloat16, tag="htp")
    nc.tensor.transpose(htp[:], helu[:], id_b[:])
    hts = pb.tile([64, 128], DT.bfloat16, tag="hts")
    nc.scalar.copy(hts[:], htp[:])
    h2ps = ps_b2.tile([128, 42], DT.float32, tag="h2ps")
    nc.tensor.matmul(h2ps[:], lhsT=hts[:], rhs=rhs2[:], start=True, stop=True)
    st2 = pb.tile([128, ROW2], DT.bfloat16, tag="st2")
    nc.scalar.copy(st2[:], h2ps[:, 0:ROW2])
    sta = pb.tile([128, 1], DT.bfloat16, tag="sta2")
    nc.scalar.copy(sta[:], h2ps[:, 41:42])
    rw = _load_rowi(nc, pb, rowi_d, b, state, "rw1")
    nc.gpsimd.indirect_dma_start(
        out=t2loc, out_offset=bass.IndirectOffsetOnAxis(ap=rw, axis=0),
        in_=st2[:], in_offset=None,
        bounds_check=meta["NPCP"] - 1, oob_is_err=False)
    nc.sync.dma_start(ad2pos[128 * b:128 * (b + 1), :], sta[:])


def _post2(nc, meta, b, blk, pb, rowi_d, out_d, b2sb, state):
    den = pb.tile([128, 1], DT.float32, tag="den2")
    nc.vector.tensor_scalar_max(den[:], blk[:, C2:C2 + 1], 1e-30)
    rec = pb.tile([128, 1], DT.float32, tag="rec2")
    nc.vector.reciprocal(rec[:], den[:])
    o2 = pb.tile([128, C2], DT.float32, tag="o2")
    nc.vector.tensor_scalar(out=o2[:], in0=blk[:, 0:C2], scalar1=rec[:],
                            scalar2=None, op0=mybir.AluOpType.mult)
    nc.vector.tensor_tensor(out=o2[:], in0=o2[:], in1=b2sb[:],
                            op=mybir.AluOpType.add)
    mx = pb.tile([128, 1], DT.float32, tag="mx")
    nc.vector.tensor_reduce(mx[:], o2[:], axis=mybir.AxisListType.X,
                            op=mybir.AluOpType.max)
    z = pb.tile([128, C2], DT.float32, tag="z")
    nc.vector.tensor_scalar(out=z[:], in0=o2[:], scalar1=mx[:], scalar2=None,
                            op0=mybir.AluOpType.subtract)
    ez = pb.tile([128, C2], DT.float32, tag="ez")
    se = pb.tile([128, 1], DT.float32, tag="se")
    nc.scalar.activation(ez[:], z[:], mybir.ActivationFunctionType.Exp,
                         accum_out=se[:])
    lse = pb.tile([128, 1], DT.float32, tag="lse")
    nc.scalar.activation(lse[:], se[:], mybir.ActivationFunctionType.Ln)
    zo = pb.tile([128, C2], DT.float32, tag="zo")
    nc.vector.tensor_scalar(out=zo[:], in0=z[:], scalar1=lse[:], scalar2=None,
                            op0=mybir.AluOpType.subtract)
    nc.sync.dma_start(out_d[128 * b:128 * (b + 1), :], zo[:])


# =================== SPMD runner (bass2jax-based, with timing) ===================

def _run_spmd(nc, in_maps, n_timing_iters=0):
    """Execute the program on NCORES neuron devices via PJRT (axon).

    Modeled on bass2jax.run_bass_via_pjrt's multi-core branch, but jits once,
    keeps inputs resident on device, and optionally times repeated runs.
    Returns (per_core_results, wall_times_s).
    """
    import jax
    from jax.sharding import Mesh, PartitionSpec
    from jax.experimental.shard_map import shard_map
    from concourse import bass2jax
    from concourse.bass2jax import _bass_exec_p, partition_id_tensor
    import time

    bass2jax.install_neuronx_cc_hook()
    assert nc.dbg_addr is None or not nc.dbg_callbacks

    in_names, out_names, out_avals, zero_outs = [], [], [], []
    partition_name = (nc.partition_id_tensor.name
                      if nc.partition_id_tensor else None)
    for alloc in nc.m.functions[0].allocations:
        if not isinstance(alloc, mybir.MemoryLocationSet):
            continue
        name = alloc.memorylocations[0].name
        if alloc.kind == "ExternalInput":
            if name != partition_name:
                in_names.append(name)
        elif alloc.kind == "ExternalOutput":
            out_names.append(name)
            shape = tuple(alloc.tensor_shape)
            dtype = mybir.dt.np(alloc.dtype)
            out_avals.append(jax.core.ShapedArray(shape, dtype))
            zero_outs.append(np.zeros(shape, dtype))
    n_params = len(in_names)
    all_in_names = in_names + out_names + (
        [partition_name] if partition_name else [])

    def _body(*args):
        operands = list(args)
        if partition_name is not None:
            operands.append(partition_id_tensor())
        return tuple(_bass_exec_p.bind(
            *operands,
            out_avals=tuple(out_avals),
            in_names=tuple(all_in_names),
            out_names=tuple(out_names),
            lowering_input_output_aliases=(),
            sim_require_finite=True,
            sim_require_nnan=True,
            nc=nc,
        ))

    devices = jax.devices()[:NCORES]
    mesh = Mesh(np.asarray(devices), ("core",))
    nin = n_params + len(out_names)
    fn = jax.jit(shard_map(_body, mesh=mesh,
                           in_specs=(PartitionSpec("core"),) * nin,
                           out_specs=(PartitionSpec("core"),) * len(out_names),
                           check_rep=False),
                 keep_unused=True)
    sh = jax.sharding.NamedSharding(mesh, PartitionSpec("core"))
    concat_in = [
        jax.device_put(np.concatenate(
            [np.asarray(in_maps[c][name]) for c in range(NCORES)], axis=0), sh)
        for name in in_names
    ]
    concat_zeros = [
        jax.device_put(np.zeros((NCORES * z.shape[0], *z.shape[1:]), z.dtype),
                       sh) for z in zero_outs
    ]
    out_arrs = jax.block_until_ready(fn(*concat_in, *concat_zeros))
    times = []
    for _ in range(n_timing_iters):
        t0 = time.perf_counter()
        r = jax.block_until_ready(fn(*concat_in, *concat_zeros))
        times.append(time.perf_counter() - t0)
        del r
    results = [
        {name: np.asarray(out_arrs[i]).reshape(NCORES, *out_avals[i].shape)[c]
         for i, name in enumerate(out_names)}
        for c in range(NCORES)
    ]
    return results, times


# =================== top-level entry ===================

def kernel(**inputs):
    K_TILES = 64
    edge_index = np.asarray(inputs["edge_index"])
    meta, per_core = preprocess(edge_index, K_TILES)
    wts = build_weight_inputs(
        np.asarray(inputs["W1"]), np.asarray(inputs["att_src1"]),
        np.asarray(inputs["att_dst1"]), np.asarray(inputs["bias1"]),
        np.asarray(inputs["W2"]), np.asarray(inputs["att_src2"]),
        np.asarray(inputs["att_dst2"]), np.asarray(inputs["bias2"]))
    x = np.asarray(inputs["x"], _f32)
    NPCP = meta["NPCP"]
    in_maps = []
    for c in range(NCORES):
        xs = np.zeros((NPCP, F), _f32)
        xs[:NPC] = x[c * NPC:(c + 1) * NPC]
        in_maps.append(dict(
            x_sl=xs, W1=wts["W1"], A1=wts["A1"], W2=wts["W2"],
            att2=wts["att2"], b1r=wts["b1r"], b2r=wts["b2r"],
            idx=per_core[c]["idx"], dloc=per_core[c]["dloc"],
            rowi=per_core[c]["rowi"], pscat=per_core[c]["pscat"]))
    nc = build_program(meta)
    n_iters = int(os.environ.get("GAT_BENCH_ITERS", "0"))
    results, times = _run_spmd(nc, in_maps, n_timing_iters=n_iters)
    global LAST_TIMES
    LAST_TIMES = times
    out = np.zeros((N, C2), _f32)
    for c in range(NCORES):
        pm = per_core[c]["posmap_flat"]
        real = pm < OOB
        out[c * NPC + pm[real]] = results[c]["out"][np.nonzero(real)[0]]
    return out
